# revision 41
# baseline (speedup 1.0000x reference)
"""Trainium2 Bass kernel for nn_AME2Encoder (dense_mlp, 8-core data parallel).

v7 strategy (on top of v5):
  - Custom PWP activation table: the Exp entry of exp_and_others is patched so
    ACT computes m(z) = elu(z)+1 exactly; every ELU site is a SINGLE
    activation pass; the +1 is folded into the next layer's bias on the host.
    Softmax uses the same function with a -24 shift.
  - conv1 contraction padded 54 -> 64 (aligned PE rows).
  - The constant pe contribution to the fuse pre-activation (wfp @ pec, same
    for every sample) is no longer a matmul: a host-precomputed [128,1024]
    tile is added into the fuse PSUM by one DVE tensor_tensor per pair-tile,
    cutting ~10%% of all PE columns.
  - Emission restructured for PE continuity: the serial query path of group g
    is emitted at the FRONT of group g+1 (overlaps a1/a2 ACTs); attention
    matmuls of group g-1 are split from their ACT/DVE finishers and
    interleaved with the global branch so the tensor engine never waits on
    the esb/softmax chain.
  - Output path: ctx @ wo done pair-major (lhsT=ctx) with a rank-1 ones
    matmul adding the bias, gf transposed on the PE; both assembled into a
    [128, 384] tile and stored with ONE DMA of 1536B-contiguous rows
    (replaces a ~150us 4-byte-scatter store tail).
"""

import os
from contextlib import ExitStack

import numpy as np

# ---- install the patched activation tables BEFORE importing concourse ----
_PWP_SRC = ('/nix/store/wxap7svlj45h0lfm31d1axjjnzyl6qsy-b16-bazel-unstable-'
            'cc-2026-05-04-9a3fa1f3-rt-2026-05-04-ade39e0a/lib/python3.13/'
            'site-packages/neuronxcc/pwp/pwp_bin_trainium/')
_PWP_DST = '/tmp/elu_pwp_v2/'


def _install_pwp():
    import shutil
    os.makedirs(_PWP_DST, exist_ok=True)
    marker = os.path.join(_PWP_DST, '.done_v2')
    if not os.path.exists(marker):
        for f in os.listdir(_PWP_SRC):
            shutil.copy(os.path.join(_PWP_SRC, f), os.path.join(_PWP_DST, f))
        raw = open(_PWP_DST + 'exp_and_others_bkt.bin', 'rb').read()
        a = np.frombuffer(raw, dtype=np.float32).reshape(941, 8).copy()
        assert (a[0:406, 4] < 0).all() and (a[406:777, 4] > 0).all()
        a[406:777, 0] = a[406:777, 4] + 1.0   # d0 = z_ref + 1
        a[406:777, 1] = 1.0                   # d1 = 1
        a[406:777, 2] = 0.0
        a[406:777, 3] = 0.0
        a[779] = [1.0, 1.0, 0.0, 0.0, 0.0, 0, 0, 0]  # pos_large: z+1
        open(_PWP_DST + 'exp_and_others_bkt.bin', 'wb').write(a.tobytes())
        open(marker, 'w').write('ok')
    os.environ['BASS_ACT_ROOT_JSON_PATH'] = _PWP_DST + 'act_info.json'


_install_pwp()

import ml_dtypes  # noqa: E402,F401

import concourse.bass as bass  # noqa: E402
import concourse.mybir as mybir  # noqa: E402
import concourse.tile as tile  # noqa: E402
from concourse.bass_utils import run_bass_kernel_spmd  # noqa: E402
from concourse.vector_clock import ScopedClock  # noqa: E402


# --- workaround: this walrus rejects the tail Drain carrying >1 sem waits ---
def _patched_dab(self, tick_clock, wait_clock):
    nc = self.nc
    probe = nc.sync.drain()
    wait_clock.add_sem_waits(probe.ins, ScopedClock({None: tick_clock.global_clock}))
    si = probe.ins.sync_info
    waits = list(si.on_wait) if si is not None else []
    if si is not None and len(waits) > 1:
        si.on_wait = waits[:1]
        for w in waits[1:]:
            n2 = nc.sync.drain()
            n2.ins.sync_info = mybir.SyncInfo(on_wait=[w], on_update=[])
    nc.all_engine_barrier()
    assert self.sems is not None
    popped = nc._tile_sem_poison_stack.pop()
    assert popped is self._sem_poison
    nc.clear_and_free_semaphores(list(self.sems.allocated().values()))
    nc.all_engine_barrier()


tile.TileContext._drain_and_barrier = _patched_dab


def _split_multiwait(nc, max_waits=1):
    """Hoist excess sem-waits onto EventSemaphore carriers."""
    ctr = [0]
    for fn in nc.m.functions:
        for blk in fn.blocks:
            insts = list(blk.instructions)
            new = []
            changed = False
            for inst in insts:
                si = inst.sync_info
                waits = list(si.on_wait) if si is not None and si.on_wait else []
                if len(waits) > max_waits:
                    changed = True
                    for w in waits[max_waits:]:
                        ctr[0] += 1
                        new.append(mybir.InstEventSemaphore(
                            name=f"zz_mw_{ctr[0]}", engine=inst.engine,
                            ins=[], outs=[],
                            sync_info=mybir.SyncInfo(on_wait=[w], on_update=[]),
                        ))
                    inst.sync_info = mybir.SyncInfo(
                        on_wait=waits[:max_waits],
                        on_update=list(si.on_update) if si.on_update else [],
                    )
                new.append(inst)
            if changed:
                blk.instructions = new


# ----- problem constants (hardcoded per spec) -----
B, C_IN, H, W = 2048, 3, 14, 36
D_LOCAL, D_POS, D_GLOBAL, D_PROP, NH = 64, 64, 128, 128, 16
HD = D_LOCAL // NH
N_CORES = 8
B_LOC = B // N_CORES      # 256
NPAIR = B_LOC // 2        # 128
NGRP = NPAIR // 4         # 32 groups of 4 pairs (8 samples)
L = H * W                 # 504
KC1 = 64                  # conv1 contraction (27+5 pad per sample, x2)
SHIFT = -24.0             # softmax shift: keeps scores in the exp region

BF = mybir.dt.float16
F32 = mybir.dt.float32
bf16 = np.float16
AX = mybir.AluOpType
AF = mybir.ActivationFunctionType


def _np_elu(x):
    return np.where(x > 0, x, np.expm1(np.minimum(x, 0.0)))


# ----------------------------------------------------------------------------
# Host-side constant packing (weight folding / layout prep)
# ----------------------------------------------------------------------------

def _block_diag2(w):
    k, m = w.shape
    out = np.zeros((2 * k, 2 * m), np.float32)
    out[:k, :m] = w
    out[k:, m:] = w
    return out


def _dup_col(b):
    return np.concatenate([b, b]).astype(np.float32)[:, None]


def host_prep_shared(inp):
    """All activations are stored as elu(z)+1; each consumer's bias absorbs
    -colsum(W) to compensate."""
    c = {}
    w1p = inp["conv1_w"].transpose(2, 3, 1, 0).reshape(27, 64)
    w1bd = np.zeros((KC1, 128), np.float32)
    w1bd[0:27, 0:64] = w1p
    w1bd[32:59, 64:128] = w1p
    c["w1bd"] = w1bd.astype(bf16)                         # [64,128]
    c["b1d"] = _dup_col(inp["conv1_b"])                   # [128,1]

    w2 = inp["conv2_w"][:, :, 0, 0].T                     # [in,out]
    c["w2bd"] = _block_diag2(w2).astype(bf16)
    c["b2d"] = _dup_col(inp["conv2_b"] - w2.sum(0))

    fl = inp["fuse_w"][:D_LOCAL]                          # [64,64]
    fp = inp["fuse_w"][D_LOCAL:]
    c["wflbd"] = _block_diag2(fl).astype(bf16)
    c["bfd"] = _dup_col(inp["fuse_b"] - fl.sum(0))

    ys = np.linspace(-1.0, 1.0, H, dtype=np.float32)
    xs = np.linspace(-1.0, 1.0, W, dtype=np.float32)
    gy, gx = np.meshgrid(ys, xs, indexing="ij")
    coords = np.stack([gx, gy], axis=-1).reshape(L, 2)
    pe = _np_elu(coords @ inp["pe_w1"] + inp["pe_b1"]) @ inp["pe_w2"] + inp["pe_b2"]
    # constant pe contribution to the fuse pre-activation, duplicated for the
    # two block-diag samples and for both 512-strided pair slots
    pet = (pe @ fp).T.astype(np.float32)                  # [64,504]
    petm = np.zeros((128, 1024), np.float32)
    petm[0:64, 0:504] = pet
    petm[64:128, 0:504] = pet
    petm[:, 512:1016] = petm[:, 0:504]
    c["petm"] = petm                                      # [128,1024] f32

    g1 = inp["g_w1"]                                      # [64,128]
    c["g1w2"] = np.vstack([g1, g1]).astype(bf16)          # [128,128]
    c["bg1d"] = (inp["g_b1"] - g1.sum(0)).astype(np.float32)[:, None]

    g2 = inp["g_w2"]
    c["g2w"] = g2.astype(bf16)
    c["bg2"] = (inp["g_b2"] - g2.sum(0)).astype(np.float32)[:, None]

    c["wvbd"] = _block_diag2(inp["wv"]).astype(bf16)
    c["wkbdT"] = _block_diag2(inp["wk"]).T.copy().astype(bf16)  # lhsT for sq2

    sm = np.zeros((64, 64), np.float32)
    for k in range(64):
        sm[k, (k // HD) * HD:(k // HD + 1) * HD] = 1.0 / np.sqrt(HD)
    c["csmbd"] = _block_diag2(sm).astype(bf16)

    c["qpwg"] = inp["qp_w"][:D_GLOBAL].astype(bf16)       # [128,64]
    c["qpwp"] = inp["qp_w"][D_GLOBAL:].astype(bf16)       # [128,64]
    c["qpb"] = inp["qp_b"].astype(np.float32)[:, None]    # [64,1]

    c["wq"] = inp["wq"].astype(bf16)
    c["bq2"] = _dup_col(inp["bq"] - inp["wq"].sum(0))     # [128,1]

    c["wo22"] = np.vstack([inp["wo"], inp["wo"]]).astype(np.float32)  # [128,64]
    cv = inp["bv"] - inp["wv"].sum(0)                     # ctx V-bias corr
    bod = inp["bo"] + cv @ inp["wo"]
    c["bodr"] = bod.astype(np.float32)[None, :]           # [1,64]
    c["ident"] = np.eye(128, dtype=np.float32)            # [128,128]
    return c


def host_prep_percore(inp):
    mf = inp["map_feat"].astype(np.float32)
    mp = np.zeros((B, 3, H + 2, W + 2), np.float32)
    mp[:, :, 1:H + 1, 1:W + 1] = mf
    from numpy.lib.stride_tricks import sliding_window_view
    sw = sliding_window_view(mp, (3, 3), axis=(2, 3))
    ic27 = sw.transpose(0, 4, 5, 1, 2, 3).reshape(B, 27, L)
    ic = np.zeros((B // 2, KC1, L), np.float32)
    ic[:, 0:27] = ic27[0::2]
    ic[:, 32:59] = ic27[1::2]
    ic = ic.astype(bf16)
    prop = inp["prop_emb"].astype(np.float32)
    cores = []
    for ci in range(N_CORES):
        sl = slice(ci * B_LOC, (ci + 1) * B_LOC)
        cores.append({
            "ic": np.ascontiguousarray(ic[ci * NPAIR:(ci + 1) * NPAIR]),
            "propT": np.ascontiguousarray(prop[sl].T).astype(bf16),
        })
    return cores


# ----------------------------------------------------------------------------
# Bass graph
# ----------------------------------------------------------------------------

def build_nc(shared):
    nc = bass.Bass()

    P = {}
    P["ic"] = nc.declare_dram_parameter("ic", [NPAIR, KC1, L], BF, isOutput=False)
    P["propT"] = nc.declare_dram_parameter("propT", [D_PROP, B_LOC], BF,
                                           isOutput=False)
    for name, arr in shared.items():
        dt = BF if arr.dtype == bf16 else F32
        P[name] = nc.declare_dram_parameter(name, list(arr.shape), dt,
                                            isOutput=False)
    OD = D_LOCAL + D_GLOBAL  # 192
    out_h = nc.declare_dram_parameter("out", [B_LOC, OD], F32, isOutput=True)

    def dram_ap(h, offset, dims):
        base = h[:]
        return bass.AP(tensor=base.tensor, offset=offset,
                       ap=[list(d) for d in dims])

    with tile.TileContext(nc) as tc, ExitStack() as ctx:
        singles = ctx.enter_context(tc.tile_pool(name="singles", bufs=1))
        p_ic = ctx.enter_context(tc.tile_pool(name="p_ic", bufs=4))
        p_act = ctx.enter_context(tc.tile_pool(name="p_act", bufs=4))
        p_pw = ctx.enter_context(tc.tile_pool(name="p_pw", bufs=2))
        p_sm = ctx.enter_context(tc.tile_pool(name="p_sm", bufs=4))
        ps = ctx.enter_context(tc.tile_pool(name="ps", bufs=2, space="PSUM"))
        pq = ctx.enter_context(tc.tile_pool(name="pq", bufs=2, space="PSUM"))

        cs = {}
        pre_ict = {}
        for name, arr in shared.items():
            dt = BF if arr.dtype == bf16 else F32
            t = singles.tile(list(arr.shape), dt, tag=f"c_{name}",
                             name=f"c_{name}")
            nc.sync.dma_start(out=t[:], in_=P[name][:])
            cs[name] = t
            if name == "w1bd":
                # prefetch group 0's inputs ahead of the remaining weight
                # constants so conv1 starts immediately
                t0 = p_ic.tile([KC1, 4 * L], BF, tag="ict", name="ict_pre0")
                for k in range(0, 4, 2):
                    d3 = t0[:, k * L:(k + 2) * L].rearrange(
                        "p (b c) -> p b c", b=2)
                    s3 = P["ic"][k:k + 2].rearrange("b p c -> p b c")
                    nc.sync.dma_start(out=d3, in_=s3)
                pre_ict[0] = t0
        t1 = p_ic.tile([KC1, 4 * L], BF, tag="ict", name="ict_pre1")
        for k in range(0, 4, 2):
            d3 = t1[:, k * L:(k + 2) * L].rearrange("p (b c) -> p b c", b=2)
            s3 = P["ic"][4 + k:4 + k + 2].rearrange("b p c -> p b c")
            nc.sync.dma_start(out=d3, in_=s3)
        pre_ict[1] = t1
        cprop = singles.tile([D_PROP, B_LOC], BF, tag="c_prop", name="c_prop")
        nc.sync.dma_start(out=cprop[:], in_=P["propT"][:])
        shiftb = singles.tile([128, 1], F32, tag="shiftb", name="shiftb")
        nc.vector.memset(shiftb[:], SHIFT)
        onesr = singles.tile([1, NPAIR], F32, tag="onesr", name="onesr")
        nc.vector.memset(onesr[:], 1.0)

        gf_all = singles.tile([D_GLOBAL, B_LOC], F32, tag="gf_all", name="gf_all")
        ctx_all = singles.tile([128, NPAIR], F32, tag="ctx_all", name="ctx_all")
        sume_all = singles.tile([128, NPAIR], F32, tag="sume_all",
                                name="sume_all")
        outT = singles.tile([128, 2 * OD], F32, tag="outT", name="outT")

        # poison psum pad columns once (both pools = all 8 banks)
        for i in range(2):
            tz = ps.tile([128, 1024], F32, tag="ps", name=f"ps_init{i}")
            nc.vector.memset(tz[:], -1.0e30)
        for i in range(2):
            tz = pq.tile([128, 1024], F32, tag="pq", name=f"pq_init{i}")
            nc.vector.memset(tz[:], -1.0e30)

        # --------- attention of a previous group, split in two phases -------
        def attn_mm(apw, asq2, q):
            """V and score matmuls for pairs 2q, 2q+1 of the pending group."""
            vt = pq.tile([128, 1024], F32, tag="pq", name=f"vsp{q}")
            st = pq.tile([128, 1024], F32, tag="pq", name=f"ssp{q}")
            for u in range(2):
                jj = 2 * q + u
                nc.tensor.matmul(vt[:, u * 512:u * 512 + L], cs["wvbd"][:],
                                 apw[:, jj * 512:jj * 512 + L],
                                 start=True, stop=True)
            for u in range(2):
                jj = 2 * q + u
                nc.tensor.matmul(st[:, u * 512:u * 512 + L],
                                 asq2[:, jj * 128:(jj + 1) * 128],
                                 apw[:, jj * 512:jj * 512 + L],
                                 start=True, stop=True)
            return vt, st

        def attn_fin(vt, st, jbase, q):
            """softmax exp + weighted-V reduction for pairs 2q, 2q+1.
            Unnormalized ctx and raw exp-sums go straight to the global
            tiles; normalization is deferred to the epilogue."""
            esb = p_sm.tile([128, 1024], F32, tag="esb", name="esb")
            nc.scalar.activation(esb[:], st[:], AF.Exp,
                                 bias=shiftb[:], scale=1.0)
            e3 = esb[:].rearrange("p (b c) -> p b c", b=2)
            nc.vector.tensor_reduce(sume_all[:, jbase + 2 * q:jbase + 2 * q + 2],
                                    e3, axis=mybir.AxisListType.X, op=AX.add)
            for u in range(2):
                wvt = p_sm.tile([128, L], F32, tag="wvt", name="wvt")
                jj = jbase + 2 * q + u
                nc.vector.scalar_tensor_tensor(
                    wvt[:], esb[:, u * 512:u * 512 + L], 1.0,
                    vt[:, u * 512:u * 512 + L], op0=AX.mult, op1=AX.mult,
                    accum_out=ctx_all[:, jj:jj + 1])

        # --------- query path of group gq (emitted one group later) ---------
        def query_path(gq):
            s0 = 8 * gq
            gsl = slice(s0, s0 + 8)
            nc.gpsimd.tensor_scalar(gf_all[:, gsl], gf_all[:, gsl],
                                    cs["bg2"][:], None, op0=AX.add)
            gfb = p_sm.tile([D_GLOBAL, 8], BF, tag="gfb", name="gfb")
            nc.vector.tensor_copy(gfb[:], gf_all[:, gsl])
            qpp = pq.tile([128, 1024], F32, tag="pq", name="qpp")
            nc.tensor.matmul(qpp[:64, 0:8], cs["qpwg"][:], gfb[:], start=True,
                             stop=False)
            nc.tensor.matmul(qpp[:64, 0:8], cs["qpwp"][:], cprop[:, gsl],
                             start=False, stop=True)
            qm = p_sm.tile([64, 8], BF, tag="qm", name="qm")
            nc.scalar.activation(qm[:], qpp[:64, 0:8], AF.Exp,
                                 bias=cs["qpb"][:], scale=1.0)
            # Q projection: even samples -> partitions 0:64, odd -> 64:128
            q_eo = qm[:].rearrange("p (j s) -> p s j", s=2)
            nc.tensor.matmul(qpp[0:64, 512:516], cs["wq"][:], q_eo[:, 0, :],
                             start=True, stop=True)
            nc.tensor.matmul(qpp[64:128, 512:516], cs["wq"][:], q_eo[:, 1, :],
                             start=True, stop=True, tile_position=(0, 64))
            Q2 = p_sm.tile([128, 4], F32, tag="Q2", name="Q2")
            nc.vector.tensor_scalar(Q2[:], qpp[:, 512:516], cs["bq2"][:],
                                    None, op0=AX.add)
            # sqbd_j = csmbd * Q2[:,j] ; sq2_j = wkbd @ sqbd_j   (K-fold)
            sqbd = p_sm.tile([128, 512], BF, tag="sqbd", name="sqbd")
            for k in range(4):
                nc.vector.tensor_scalar(sqbd[:, k * 128:(k + 1) * 128],
                                        cs["csmbd"][:], Q2[:, k:k + 1], None,
                                        op0=AX.mult)
            sq2p = pq.tile([128, 1024], F32, tag="pq", name="sq2p")
            nc.tensor.matmul(sq2p[:, 0:512], cs["wkbdT"][:], sqbd[:, 0:512],
                             start=True, stop=True)
            sq2 = p_sm.tile([128, 512], BF, tag="sq2", name="sq2")
            nc.vector.tensor_copy(sq2[:], sq2p[:, 0:512])
            nc.vector.memset(sq2p[:, 504:512], -1.0e30)
            return sq2

        def attn_all(apw, asq2, gprev):
            """Full attention of group gprev (epilogue use)."""
            for q in range(2):
                vt, st = attn_mm(apw, asq2, q)
                attn_fin(vt, st, 4 * gprev, q)

        pw_prev = None
        for g in range(NGRP):
            j0 = 4 * g          # first pair of group
            s0 = 8 * g          # first sample of group

            # ---- ic load (4 pairs, packed; groups 0/1 prefetched) ----
            if g in pre_ict:
                ict = pre_ict.pop(g)
            else:
                ict = p_ic.tile([KC1, 4 * L], BF, tag="ict", name="ict")
                for k in range(0, 4, 2):
                    d3 = ict[:, k * L:(k + 2) * L].rearrange(
                        "p (b c) -> p b c", b=2)
                    s3 = P["ic"][j0 + k:j0 + k + 2].rearrange("b p c -> p b c")
                    nc.sync.dma_start(out=d3, in_=s3)

            # ---- conv1 ----
            pt, a1t, a2t = {}, {}, {}
            for q in range(2):
                t = ps.tile([128, 1024], F32, tag="ps", name="c1p")
                for u in range(2):
                    nc.tensor.matmul(t[:, u * 512:u * 512 + L], cs["w1bd"][:],
                                     ict[:, (2 * q + u) * L:(2 * q + u + 1) * L],
                                     start=True, stop=True)
                pt[q] = t

            # ---- deferred query path of the previous group ----
            prev_sq2 = query_path(g - 1) if g > 0 else None

            # ---- conv2 / fuse ----
            for q in range(2):
                a1 = p_act.tile([128, 1024], BF, tag="a1", name="a1")
                nc.scalar.activation(a1[:], pt[q][:], AF.Exp,
                                     bias=cs["b1d"][:], scale=1.0)
                a1t[q] = a1
            for q in range(2):
                t = ps.tile([128, 1024], F32, tag="ps", name="c2p")
                for u in range(2):
                    nc.tensor.matmul(t[:, u * 512:u * 512 + L], cs["w2bd"][:],
                                     a1t[q][:, u * 512:u * 512 + L],
                                     start=True, stop=True)
                pt[q] = t
            for q in range(2):
                a2 = p_act.tile([128, 1024], BF, tag="a2", name="a2")
                nc.scalar.activation(a2[:], pt[q][:], AF.Exp,
                                     bias=cs["b2d"][:], scale=1.0)
                a2t[q] = a2
            for q in range(2):
                t = ps.tile([128, 1024], F32, tag="ps", name="fp")
                for u in range(2):
                    nc.tensor.matmul(t[:, u * 512:u * 512 + L], cs["wflbd"][:],
                                     a2t[q][:, u * 512:u * 512 + L],
                                     start=True, stop=True)
                pt[q] = t
            petv = cs["petm"][:].rearrange("p (b c) -> p b c", b=2)[:, :, 0:L]
            for q in range(2):
                f3 = pt[q][:].rearrange("p (b c) -> p b c", b=2)[:, :, 0:L]
                nc.vector.tensor_tensor(f3, f3, petv, op=AX.add)
            pw = p_pw.tile([128, 2048], BF, tag="pw", name="pw")
            for q in range(2):
                nc.scalar.activation(pw[:, q * 1024:(q + 1) * 1024], pt[q][:],
                                     AF.Exp, bias=cs["bfd"][:], scale=1.0)

            # ---- global branch interleaved with attention of group g-1 ----
            have_attn = prev_sq2 is not None
            if have_attn:
                vst = {0: attn_mm(pw_prev, prev_sq2, 0)}

            for q in range(2):          # q-major: head-of-line waits pw(q0)
                g1pt = {}
                for par in range(2):
                    g1pt[par] = ps.tile([128, 1024], F32, tag="ps", name="g1p")
                for par in range(2):
                    for u in range(2):
                        jj = 2 * q + u
                        nc.tensor.matmul(
                            g1pt[par][:, u * 512:u * 512 + L],
                            cs["g1w2"][par * 64:(par + 1) * 64, :],
                            pw[par * 64:(par + 1) * 64, jj * 512:jj * 512 + L],
                            start=True, stop=True)
                if have_attn and q == 1:
                    vst[1] = attn_mm(pw_prev, prev_sq2, 1)
                for par in range(2):
                    g1a = p_act.tile([128, 1024], BF, tag="g1a", name="g1a")
                    nc.scalar.activation(g1a[:], g1pt[par][:],
                                         AF.Exp, bias=cs["bg1d"][:], scale=1.0)
                    t = ps.tile([128, 1024], F32, tag="ps", name="g2p")
                    for u in range(2):
                        nc.tensor.matmul(t[:, u * 512:u * 512 + L], cs["g2w"][:],
                                         g1a[:, u * 512:u * 512 + L],
                                         start=True, stop=True)
                    # samples s0+4q+par and s0+4q+par+2
                    t3 = t[:].rearrange("p (b c) -> p b c", b=2)
                    gfv = gf_all[:, s0 + 4 * q:s0 + 4 * q + 4].rearrange(
                        "p (x two) -> p two x", two=2)
                    nc.vector.tensor_reduce(gfv[:, par, :], t3,
                                            axis=mybir.AxisListType.X,
                                            op=AX.max)
                if have_attn:
                    attn_fin(vst[q][0], vst[q][1], 4 * (g - 1), q)
            pw_prev = pw

        # ============== epilogue: last query + last attention ===============
        sq2_last = query_path(NGRP - 1)
        attn_all(pw_prev, sq2_last, NGRP - 1)

        # ================= output projection + transposed store =============
        # batched softmax normalization: per-(head-dim, pair), once at the end
        rec_all = p_sm.tile([128, NPAIR], F32, tag="rec_all", name="rec_all")
        nc.vector.reciprocal(rec_all[:], sume_all[:])
        nc.vector.tensor_tensor(ctx_all[:], ctx_all[:], rec_all[:],
                                op=AX.mult)
        # wl^T: out[pair, dim] = sum_d ctx[d, pair] * wo[d, dim] + bod[dim]
        wlp = ps.tile([128, 1024], F32, tag="ps", name="wlp")
        for par in range(2):
            sl = slice(par * 64, par * 64 + 64)
            psl = slice(par * 64, (par + 1) * 64)
            nc.tensor.matmul(wlp[:, sl], ctx_all[psl, :],
                             cs["wo22"][psl, :], start=True, stop=False)
            nc.tensor.matmul(wlp[:, sl], onesr[:], cs["bodr"][:],
                             start=False, stop=True)
        # gf^T by parity: even samples -> [pair, 128], odd -> [pair, 128]
        gftp = ps.tile([128, 1024], F32, tag="ps", name="gftp")
        gf_eo = gf_all[:].rearrange("p (j two) -> p two j", two=2)
        for par in range(2):
            nc.tensor.matmul(gftp[:, par * 128:(par + 1) * 128],
                             gf_eo[:, par, :], cs["ident"][:],
                             is_transpose=True)
        # assemble [pair, 2*OD] then one contiguous store
        for par in range(2):
            nc.vector.tensor_copy(outT[:, par * OD:par * OD + 64],
                                  wlp[:, par * 64:par * 64 + 64])
            nc.vector.tensor_copy(outT[:, par * OD + 64:par * OD + OD],
                                  gftp[:, par * 128:(par + 1) * 128])
        nc.sync.dma_start(out=dram_ap(out_h, 0, [[2 * OD, NPAIR], [1, 2 * OD]]),
                          in_=outT[:])

    _split_multiwait(nc, max_waits=int(os.environ.get("AME2_MAXWAITS", "1")))
    return nc
